# revision 1
# baseline (speedup 1.0000x reference)
import sys

sys.path.insert(0, "/opt/trn_rl_repo")

import numpy as np
from contextlib import ExitStack
from concourse import bacc, bass_utils, tile, mybir

F32 = mybir.dt.float32
F32R = mybir.dt.float32r
TANH = mybir.ActivationFunctionType.Tanh
ADD = mybir.AluOpType.add

NCORES = 8
USE_W2LO = False


def _r11(x):
    # round to fp32r (11 explicit mantissa bits) so device-side fp32r
    # rounding of these constants is an exact identity
    i = np.ascontiguousarray(x, dtype=np.float32).view(np.int32)
    i = (i + 0x800) & ~0xFFF
    return i.view(np.float32)


def _build(nu, d_idx, pairs, steps, ncores):
    # Per-core layout: batch N = pairs*1024, y tile [8, W=pairs*512]:
    # partition p = 4*dup + 2*AB + c, col = pr*512 + i, sample
    # n = pr*1024 + AB*512 + i. dup duplicates y so mm1 applies W1hi
    # (K rows 0-3) and W1lo (K rows 4-7) in one pass. fp32r matmul dst
    # must start at partition 0, so dy goes to PSUM base 0 and the
    # Euler add happens on DVE. The state itself must stay fp32
    # (rounding the state each step costs 100x in accuracy), so mm1
    # reads a separately-maintained f32r shadow of y.
    W = pairs * 512
    G = pairs // 2
    M = pairs * 8
    HB = 3

    nc = bacc.Bacc(
        "TRN2",
        target_bir_lowering=False,
        debug=False,
        enable_asserts=False,
        num_devices=ncores,
    )
    y0pre = nc.dram_tensor("y0pre", [8, W], F32, kind="ExternalInput")
    w1bd_d = nc.dram_tensor("w1bd", [8, 100], F32, kind="ExternalInput")
    b1bd_d = nc.dram_tensor("b1bd", [100, 1], F32, kind="ExternalInput")
    w2hi_d = nc.dram_tensor("w2hi", [100, 8 * nu], F32, kind="ExternalInput")
    w2lo_d = nc.dram_tensor("w2lo", [100, 8 * nu], F32, kind="ExternalInput")
    dtb2_d = nc.dram_tensor("dtb2", [8, nu], F32, kind="ExternalInput")
    out_d = nc.dram_tensor("out", [steps, 128, 2 * M], F32, kind="ExternalOutput")
    planar_d = nc.dram_tensor("planar", [steps, 2, pairs * 1024], F32)

    with tile.TileContext(nc) as tc:
        with ExitStack() as ctx:
            sb = ctx.enter_context(tc.tile_pool(name="sb", bufs=1, space="SBUF"))
            ps = ctx.enter_context(tc.tile_pool(name="ps", bufs=1, space="PSUM"))

            w1_sb = sb.tile([8, 100], F32R, tag="w1", name="w1_sb")
            b1_sb = sb.tile([100, 1], F32, tag="b1", name="b1_sb")
            w2hi_sb = sb.tile([100, 8 * nu], F32R, tag="w2h", name="w2hi_sb")
            w2lo_sb = sb.tile([100, 8 * nu], F32R, tag="w2l", name="w2lo_sb")
            dtb2_sb = sb.tile([8, nu], F32, tag="db2", name="dtb2_sb")
            st_w1 = sb.tile([8, 100], F32, tag="sw1", name="st_w1")
            st_w2h = sb.tile([100, 8 * nu], F32, tag="sw2h", name="st_w2h")
            st_w2l = sb.tile([100, 8 * nu], F32, tag="sw2l", name="st_w2l")
            st_y0 = sb.tile([8, W], F32, tag="sy0", name="st_y0")
            y_bufs = [
                sb.tile([8, W], F32, tag=f"y{i}", name=f"y{i}") for i in range(2)
            ]
            yr_bufs = [
                sb.tile([8, W], F32R, tag=f"yr{i}", name=f"yr{i}") for i in range(2)
            ]
            h_bufs = [
                sb.tile([100, 1024], F32R, tag=f"h{i}", name=f"h{i}")
                for i in range(HB)
            ]
            p_bufs = [
                sb.tile([128, 2 * M], F32, tag=f"p{i}", name=f"p{i}") for i in range(2)
            ]
            f_bufs = [
                sb.tile([128, 2 * M], F32, tag=f"f{i}", name=f"f{i}") for i in range(2)
            ]
            a_bufs = [
                ps.tile([128, 1024], F32, tag=f"a{i}", name=f"a{i}") for i in range(2)
            ]
            dy_bufs = [
                ps.tile([8, 1024], F32, tag=f"d{i}", name=f"d{i}") for i in range(2)
            ]

            nc.sync.dma_start(out=st_w1[:, :], in_=w1bd_d[:, :])
            nc.sync.dma_start(out=b1_sb[:, :], in_=b1bd_d[:, :])
            nc.sync.dma_start(out=st_w2h[:, :], in_=w2hi_d[:, :])
            nc.sync.dma_start(out=st_w2l[:, :], in_=w2lo_d[:, :])
            nc.sync.dma_start(out=dtb2_sb[:, :], in_=dtb2_d[:, :])
            nc.sync.dma_start(out=st_y0[:, :], in_=y0pre[:, :])
            nc.vector.tensor_copy(out=w1_sb[:, :], in_=st_w1[:, :])
            nc.vector.tensor_copy(out=w2hi_sb[:, :], in_=st_w2h[:, :])
            nc.vector.tensor_copy(out=w2lo_sb[:, :], in_=st_w2l[:, :])
            nc.vector.tensor_copy(out=y_bufs[0][:, :], in_=st_y0[:, :])
            nc.vector.tensor_copy(out=yr_bufs[0][:, :], in_=st_y0[:, :])

            for s in range(steps):
                di = d_idx[s]
                y_cur = y_bufs[s % 2]
                y_nxt = y_bufs[(s + 1) % 2]
                yr_cur = yr_bufs[s % 2]
                yr_nxt = yr_bufs[(s + 1) % 2]
                for g in range(G):
                    cs = slice(g * 1024, (g + 1) * 1024)
                    a = a_bufs[g % 2]
                    # matmul dst cannot cross a PSUM bank (512 fp32), so
                    # each 1024-wide chunk is two 512-wide matmuls
                    for half in (0, 1):
                        hs = slice(half * 512, (half + 1) * 512)
                        nc.tensor.matmul(
                            a[0:100, hs],
                            lhsT=w1_sb[:, :],
                            rhs=yr_cur[:, g * 1024 + half * 512 : g * 1024 + (half + 1) * 512],
                            start=True,
                            stop=True,
                            tile_position=(0, 0),
                        )
                    h = h_bufs[(s * G + g) % HB]
                    nc.scalar.activation(
                        h[:, :], a[0:100, :], TANH, bias=b1_sb[:, :]
                    )
                    dy = dy_bufs[g % 2]
                    for half in (0, 1):
                        hs = slice(half * 512, (half + 1) * 512)
                        nc.tensor.matmul(
                            dy[:, hs],
                            lhsT=w2hi_sb[:, 8 * di : 8 * di + 8],
                            rhs=h[:, hs],
                            start=True,
                            stop=not USE_W2LO,
                            tile_position=(0, 0),
                        )
                        if USE_W2LO:
                            nc.tensor.matmul(
                                dy[:, hs],
                                lhsT=w2lo_sb[:, 8 * di : 8 * di + 8],
                                rhs=h[:, hs],
                                start=False,
                                stop=True,
                                tile_position=(0, 0),
                            )
                    # y_nxt = (dy + dt*b2) + y_cur in full fp32
                    nc.vector.scalar_tensor_tensor(
                        out=y_nxt[:, cs],
                        in0=dy[:, :],
                        scalar=dtb2_sb[:, di : di + 1],
                        in1=y_cur[:, cs],
                        op0=ADD,
                        op1=ADD,
                    )
                    # f32r shadow for next step's mm1; split between ACT
                    # and DVE to balance engine load
                    if g % 2 == 0:
                        nc.scalar.copy(out=yr_nxt[:, cs], in_=y_nxt[:, cs])
                    else:
                        nc.vector.tensor_copy(out=yr_nxt[:, cs], in_=y_nxt[:, cs])
                # planar store: [c, pr, i] <- y_nxt[2*ab+c, pr*512+i], per ab
                for ab in (0, 1):
                    nc.sync.dma_start(
                        out=planar_d[s].rearrange(
                            "c (pr ab i) -> c ab pr i", pr=pairs, ab=2, i=512
                        )[:, ab],
                        in_=y_nxt[2 * ab : 2 * ab + 2, :].rearrange(
                            "c (pr i) -> c pr i", pr=pairs, i=512
                        ),
                    )
                # phase 2: reload planar as [q, c, m], interleave to [q, 2m+c]
                p2 = p_bufs[s % 2]
                nc.sync.dma_start(
                    out=p2[:, :].rearrange("q (c m) -> q c m", c=2, m=M),
                    in_=planar_d[s].rearrange("c (q m) -> q c m", q=128, m=M),
                )
                f2 = f_bufs[s % 2]
                fv = f2[:, :].rearrange("q (m c) -> q c m", m=M, c=2)
                nc.vector.tensor_copy(out=fv[:, 0, :], in_=p2[:, 0:M])
                nc.vector.tensor_copy(out=fv[:, 1, :], in_=p2[:, M : 2 * M])
                nc.sync.dma_start(out=out_d[s], in_=f2[:, :])
    nc.compile()
    return nc


def _prep(y0, t, w1, b1, w2, b2, ncores):
    B = y0.shape[0]
    steps = t.shape[0] - 1
    N = B // ncores
    pairs = N // 1024
    dts = (t[1:] - t[:-1]).astype(np.float32)
    uniq, inv = np.unique(dts, return_inverse=True)
    nu = len(uniq)
    w1hi = _r11(w1)
    w1lo = _r11((w1 - w1hi).astype(np.float32))
    w1bd = np.zeros((8, 100), np.float32)
    w1bd[0:2, 0:50] = w1hi.T
    w1bd[2:4, 50:100] = w1hi.T
    w1bd[4:6, 0:50] = w1lo.T
    w1bd[6:8, 50:100] = w1lo.T
    b1bd = np.concatenate([b1, b1]).astype(np.float32).reshape(100, 1)
    w2hi = np.zeros((100, 8 * nu), np.float32)
    w2lo = np.zeros((100, 8 * nu), np.float32)
    dtb2 = np.zeros((8, nu), np.float32)
    for d in range(nu):
        dw2 = (uniq[d] * w2).astype(np.float32)
        db2 = (uniq[d] * b2).astype(np.float32)
        hi = _r11(dw2)
        lo = _r11((dw2 - hi).astype(np.float32))
        for dup in (0, 1):
            o = 8 * d + 4 * dup
            w2hi[0:50, o : o + 2] = hi.T
            w2hi[50:100, o + 2 : o + 4] = hi.T
            w2lo[0:50, o : o + 2] = lo.T
            w2lo[50:100, o + 2 : o + 4] = lo.T
        dtb2[:, d] = [db2[0], db2[1], db2[0], db2[1]] * 2
    in_maps = []
    for k in range(ncores):
        yk4 = (
            y0[k * N : (k + 1) * N]
            .reshape(pairs, 2, 512, 2)
            .transpose(1, 3, 0, 2)
            .reshape(4, pairs * 512)
        )
        yk = np.concatenate([yk4, yk4], axis=0).copy()
        in_maps.append(
            {
                "y0pre": yk,
                "w1bd": w1bd,
                "b1bd": b1bd,
                "w2hi": w2hi,
                "w2lo": w2lo,
                "dtb2": dtb2,
            }
        )
    return nu, list(inv), pairs, steps, N, in_maps


def run(y0, t, w1, b1, w2, b2, ncores=NCORES, steps_override=None, trace=False):
    y0 = np.ascontiguousarray(y0, dtype=np.float32)
    nu, inv, pairs, steps, N, in_maps = _prep(
        y0, np.asarray(t), np.asarray(w1), np.asarray(b1), np.asarray(w2),
        np.asarray(b2), ncores,
    )
    if steps_override is not None:
        steps = steps_override
    nc = _build(nu, inv, pairs, steps, ncores)
    res = bass_utils.run_bass_kernel_spmd(
        nc, in_maps, list(range(ncores)), trace=trace
    )
    B = y0.shape[0]
    out = np.empty((steps + 1, B, 2), np.float32)
    out[0] = y0
    for k in range(ncores):
        out[1:, k * N : (k + 1) * N, :] = (
            np.asarray(res.results[k]["out"]).reshape(steps, N, 2)
        )
    return out, res


def kernel(**inputs):
    out, _ = run(
        inputs["y0"], inputs["t"], inputs["w1"], inputs["b1"], inputs["w2"],
        inputs["b2"],
    )
    return out



# revision 10
# speedup vs baseline: 86.7824x; 86.7824x over previous
import sys

sys.path.insert(0, "/opt/trn_rl_repo")

import numpy as np
from contextlib import ExitStack
from concourse import bacc, bass_utils, tile, mybir

F32 = mybir.dt.float32
F32R = mybir.dt.float32r
BF16 = mybir.dt.bfloat16
TANH = mybir.ActivationFunctionType.Tanh

NCORES = 8


def _r11(x):
    # round to fp32r (11 explicit mantissa bits) so device-side fp32r
    # rounding of these constants is an exact identity
    i = np.ascontiguousarray(x, dtype=np.float32).view(np.int32)
    i = (i + 0x800) & ~0xFFF
    return i.view(np.float32)


def _rbf(x):
    # round fp32 to bf16 (round-to-nearest-even), kept in fp32 storage
    i = np.ascontiguousarray(x, dtype=np.float32).view(np.uint32)
    i = (i + 0x7FFF + ((i >> 16) & 1)) & 0xFFFF0000
    return i.view(np.float32)


def _build(nu, d_idx, steps, ncores):
    # Per-core layout: N = 16384 samples in 16 chunks of 512 cols
    # (2 samples/col). y state lives permanently in PSUM fp32 (banks
    # 0-3): chunk c -> partition strip 32*(c%4)+[0..8), cols
    # 512*(c//4)+[0..512). Rows within a strip: p = 4*dup + 2*AB + c
    # (dup duplicates so one K=8 mm1 applies W1hi+W1lo). Each Euler
    # step, mm2 (bf16, col-tiled at 32*(c%4), hi+lo weight pair)
    # ACCUMULATES dt*(W2 h + b2) onto y via start=False, so no vector
    # Euler add is needed. The state recirculates PSUM->SBUF (f32r)
    # via DVE/ACT copies (DMA and gpsimd can't read PSUM). mm1 stays
    # f32r (its dst is at partition 0; f32r can't col-tile) and is
    # 4-way row-tiled at 32*(c%4).
    nc = bacc.Bacc(
        "TRN2",
        target_bir_lowering=False,
        debug=False,
        enable_asserts=False,
        num_devices=ncores,
    )
    W = 8192  # 16 chunks * 512
    w1rep_d = nc.dram_tensor("w1rep", [128, 100], F32, kind="ExternalInput")
    b1bd_d = nc.dram_tensor("b1bd", [100, 1], F32, kind="ExternalInput")
    w2f_d = nc.dram_tensor("w2f", [101, 16 * nu], F32, kind="ExternalInput")
    eye12_d = nc.dram_tensor("eye12", [12, 8], F32, kind="ExternalInput")
    y0pre_d = nc.dram_tensor("y0pre", [12, W], F32, kind="ExternalInput")
    out_d = nc.dram_tensor("out", [steps, 4, W], F32, kind="ExternalOutput")

    HB = 3
    with tile.TileContext(nc) as tc:
        with ExitStack() as ctx:
            sb = ctx.enter_context(tc.tile_pool(name="sb", bufs=1, space="SBUF"))
            ps = ctx.enter_context(tc.tile_pool(name="ps", bufs=1, space="PSUM"))

            w1_sb = sb.tile([128, 100], F32R, tag="w1", name="w1_sb")
            b1_sb = sb.tile([100, 1], F32, tag="b1", name="b1_sb")
            w2_sb = sb.tile([101, 16 * nu], BF16, tag="w2", name="w2_sb")
            eye_sb = sb.tile([12, 8], BF16, tag="eye", name="eye_sb")
            y0_sb = sb.tile([12, W], BF16, tag="y0", name="y0_sb")
            st_w1 = sb.tile([128, 100], F32, tag="sw1", name="st_w1")
            st_w2 = sb.tile([101, 16 * nu], F32, tag="sw2", name="st_w2")
            st_eye = sb.tile([12, 8], F32, tag="sey", name="st_eye")
            st_y0 = sb.tile([12, W], F32, tag="sy0", name="st_y0")
            yr = sb.tile([128, 2048], F32R, tag="yr", name="yr")
            h_bufs = [
                sb.tile([101, 1024], BF16, tag=f"h{i}", name=f"h{i}")
                for i in range(HB)
            ]
            y_ps = ps.tile([128, 2048], F32, tag="y", name="y_ps")
            a_bufs = [
                ps.tile([128, 1024], F32, tag=f"a{i}", name=f"a{i}") for i in range(2)
            ]

            nc.sync.dma_start(out=st_w1[:, :], in_=w1rep_d[:, :])
            nc.sync.dma_start(out=b1_sb[:, :], in_=b1bd_d[:, :])
            nc.sync.dma_start(out=st_w2[:, :], in_=w2f_d[:, :])
            nc.sync.dma_start(out=st_eye[:, :], in_=eye12_d[:, :])
            nc.sync.dma_start(out=st_y0[:, :], in_=y0pre_d[:, :])
            nc.vector.tensor_copy(out=w1_sb[:, :], in_=st_w1[:, :])
            nc.vector.tensor_copy(out=w2_sb[:, :], in_=st_w2[:, :])
            nc.vector.tensor_copy(out=eye_sb[:, :], in_=st_eye[:, :])
            nc.vector.tensor_copy(out=y0_sb[:, :], in_=st_y0[:, :])

            # h row 100 is a constant-1 bias row (so mm2 adds dt*b2 via
            # lhsT row 100). memset can't write BF16 reliably and engine
            # partition bases must be 32-aligned, so stage rows 96-101 in
            # F32 and copy; rows 96-99 get overwritten by every tanh.
            ones_st = sb.tile([101, 1024], F32, tag="one", name="ones_st")
            nc.vector.memset(ones_st[96:101, :], 1.0)
            for i in range(HB):
                nc.vector.tensor_copy(
                    out=h_bufs[i][96:101, :], in_=ones_st[96:101, :]
                )

            # init: y_psum[chunk] = y0hi + y0mid + y0lo via identity matmul
            for c in range(16):
                j, b = c % 4, c // 4
                nc.tensor.matmul(
                    y_ps[32 * j : 32 * j + 8, 512 * b : 512 * b + 512],
                    lhsT=eye_sb[:, :],
                    rhs=y0_sb[:, 512 * c : 512 * c + 512],
                    start=True,
                    stop=True,
                    tile_position=(0, 32 * j),
                )

            def recirc(j):
                # PSUM y (fp32) -> SBUF yr (f32r) for next step's mm1;
                # DMA can't read PSUM and gpsimd can't either, so split the
                # 4x2048 cols between DVE (bulk) and ACT (tail) for balance
                src = y_ps[32 * j : 32 * j + 8, :]
                dst = yr[32 * j : 32 * j + 8, :]
                if j == 3:
                    nc.vector.tensor_copy(out=dst[:, 0:1536], in_=src[:, 0:1536])
                    nc.scalar.copy(out=dst[:, 1536:2048], in_=src[:, 1536:2048])
                else:
                    nc.vector.tensor_copy(out=dst, in_=src)

            for j in range(4):
                recirc(j)

            for s in range(steps):
                di = d_idx[s]
                for k in range(4):  # round k: chunks 4k..4k+3, strips q=0..3
                    cs = slice(512 * k, 512 * k + 512)
                    for q in range(4):
                        nc.tensor.matmul(
                            a_bufs[q // 2][0:100, 512 * (q % 2) : 512 * (q % 2) + 512],
                            lhsT=w1_sb[32 * q : 32 * q + 8, :],
                            rhs=yr[32 * q : 32 * q + 8, cs],
                            start=True,
                            stop=True,
                            tile_position=(32 * q, 0),
                        )
                    for half in range(2):
                        h = h_bufs[(2 * k + half) % HB]
                        nc.scalar.activation(
                            h[0:100, :],
                            a_bufs[half][0:100, :],
                            TANH,
                            bias=b1_sb[:, :],
                        )
                    for q in range(4):
                        h = h_bufs[(2 * k + q // 2) % HB]
                        hs = h[:, 512 * (q % 2) : 512 * (q % 2) + 512]
                        for part in range(2):  # hi then lo weights
                            nc.tensor.matmul(
                                y_ps[32 * q : 32 * q + 8, cs],
                                lhsT=w2_sb[
                                    :, 16 * di + 8 * part : 16 * di + 8 * part + 8
                                ],
                                rhs=hs,
                                start=False,
                                stop=True,
                                tile_position=(0, 32 * q),
                                skip_group_check=True,
                            )
                for j in range(4):
                    recirc(j)
                    nc.sync.dma_start(
                        out=out_d[s].rearrange(
                            "p (b j2 i) -> p b j2 i", b=4, j2=4, i=512
                        )[:, :, j],
                        in_=yr[32 * j : 32 * j + 4, :]
                        .bitcast(F32)
                        .rearrange("p (b i) -> p b i", b=4, i=512),
                    )
    nc.compile()
    return nc


def _prep(y0, t, w1, b1, w2, b2, ncores):
    B = y0.shape[0]
    steps = t.shape[0] - 1
    N = B // ncores
    dts = (t[1:] - t[:-1]).astype(np.float32)
    uniq, inv = np.unique(dts, return_inverse=True)
    nu = len(uniq)
    w1hi = _r11(w1)
    w1lo = _r11((w1 - w1hi).astype(np.float32))
    w1bd = np.zeros((8, 100), np.float32)
    w1bd[0:2, 0:50] = w1hi.T
    w1bd[2:4, 50:100] = w1hi.T
    w1bd[4:6, 0:50] = w1lo.T
    w1bd[6:8, 50:100] = w1lo.T
    w1rep = np.zeros((128, 100), np.float32)
    for j in range(4):
        w1rep[32 * j : 32 * j + 8] = w1bd
    b1bd = np.concatenate([b1, b1]).astype(np.float32).reshape(100, 1)
    w2f = np.zeros((101, 16 * nu), np.float32)
    for d in range(nu):
        dw2 = (uniq[d] * w2).astype(np.float32)
        db2 = (uniq[d] * b2).astype(np.float32)
        hi2 = _rbf(dw2)
        lo2 = _rbf((dw2 - hi2).astype(np.float32))
        bhi = _rbf(db2)
        blo = _rbf((db2 - bhi).astype(np.float32))
        for part, (wv, bv) in enumerate(((hi2, bhi), (lo2, blo))):
            o0 = 16 * d + 8 * part
            for dup in (0, 1):
                o = o0 + 4 * dup
                w2f[0:50, o : o + 2] = wv.T
                w2f[50:100, o + 2 : o + 4] = wv.T
            w2f[100, o0 : o0 + 8] = [bv[0], bv[1], bv[0], bv[1]] * 2
    eye12 = np.zeros((12, 8), np.float32)
    for r in range(12):
        for m in range(8):
            if r % 4 == m % 4:
                eye12[r, m] = 1.0
    y0hi = _rbf(y0)
    y0mid = _rbf((y0 - y0hi).astype(np.float32))
    y0lo = _rbf((y0 - y0hi - y0mid).astype(np.float32))
    in_maps = []
    for k in range(ncores):
        yk = np.empty((12, N // 2), np.float32)
        for src, base in ((y0hi, 0), (y0mid, 4), (y0lo, 8)):
            # row 2*AB+c, col 512*chunk+i = src[kN + 1024*chunk + 512*AB + i, c]
            blk = src[k * N : (k + 1) * N].reshape(16, 2, 512, 2)  # chunk,AB,i,c
            yk[base : base + 4] = (
                blk.transpose(1, 3, 0, 2).reshape(4, N // 2)
            )
        in_maps.append(
            {
                "w1rep": w1rep,
                "b1bd": b1bd,
                "w2f": w2f,
                "eye12": eye12,
                "y0pre": yk,
            }
        )
    return nu, list(inv), steps, N, in_maps


def run(y0, t, w1, b1, w2, b2, ncores=NCORES, steps_override=None, trace=False):
    y0 = np.ascontiguousarray(y0, dtype=np.float32)
    nu, inv, steps, N, in_maps = _prep(
        y0, np.asarray(t), np.asarray(w1), np.asarray(b1), np.asarray(w2),
        np.asarray(b2), ncores,
    )
    if steps_override is not None:
        steps = steps_override
    nc = _build(nu, inv, steps, ncores)
    res = bass_utils.run_bass_kernel_spmd(
        nc, in_maps, list(range(ncores)), trace=trace
    )
    B = y0.shape[0]
    out = np.empty((steps + 1, B, 2), np.float32)
    out[0] = y0
    for k in range(ncores):
        v = np.asarray(res.results[k]["out"])  # [steps, 4, 8192]
        v = v.reshape(steps, 2, 2, 16, 512)  # s, AB, c, chunk, i
        out[1:, k * N : (k + 1) * N, :] = (
            v.transpose(0, 3, 1, 4, 2).reshape(steps, N, 2)
        )
    return out, res


def kernel(**inputs):
    out, _ = run(
        inputs["y0"], inputs["t"], inputs["w1"], inputs["b1"], inputs["w2"],
        inputs["b2"],
    )
    return out
